# revision 22
# baseline (speedup 1.0000x reference)
"""Multi-head masked attention on 8 Trainium2 NeuronCores.

Sharding: data-parallel over batch (B=2 -> 2 groups of 4 cores),
tensor-parallel over heads within a group (16 heads -> 4 heads/core).
Each core computes q/k/v projections for its 4 heads (column-sharded),
causal attention in the transposed (S^T) domain, and a row-sharded
partial o-projection. The host sums the 4 partials per batch element
and adds the output bias.

All matmul operands are bf16 (PSUM accumulation stays f32): bf16
draws less power than f32r under the HAM duty-cycle throttle. x is
pre-transposed on the host so x^T streams straight from HBM with no
PE transpose pass. The causal mask is accumulated into PSUM by the PE
(identity-stationary matmul streaming a triangular constant), and the
diagonal QK/exp work is restricted to the unmasked columns.

Self-contained: hardcodes shapes B=2, T=2048, C=1024, H=16, Dh=64.
"""

import sys

sys.path.insert(0, "/opt/trn_rl_repo")

import numpy as np

import concourse.bass as bass
import concourse.tile as tile
import concourse.mybir as mybir
from concourse import bacc
from concourse.bass import ts, ds
from concourse.masks import make_identity, make_lower_triangular

F32 = mybir.dt.float32
BF16 = mybir.dt.bfloat16
AF = mybir.ActivationFunctionType
ALU = mybir.AluOpType

B, T, C = 2, 2048, 1024
H, DH = 16, 64
HPC = 4            # heads per core
DQC = HPC * DH     # 256 projected dims per core
N_CORES = 8
NEG = -1.0e30


def build_program():
    nc = bacc.Bacc("TRN2", target_bir_lowering=False, debug=False)

    xbT = nc.dram_tensor("xbT", [C, T], BF16, kind="ExternalInput")
    wq = nc.dram_tensor("wq", [C, DQC], BF16, kind="ExternalInput")
    wk = nc.dram_tensor("wk", [C, DQC], BF16, kind="ExternalInput")
    wv = nc.dram_tensor("wv", [C, DQC], BF16, kind="ExternalInput")
    wo = nc.dram_tensor("wo", [DQC, C], BF16, kind="ExternalInput")
    bq = nc.dram_tensor("bq", [DQC], F32, kind="ExternalInput")
    bk = nc.dram_tensor("bk", [DQC], F32, kind="ExternalInput")
    bv = nc.dram_tensor("bv", [DQC], F32, kind="ExternalInput")
    out = nc.dram_tensor("out", [T, C], BF16, kind="ExternalOutput")

    TC = T // 128    # 16 t-chunks of 128
    CC = C // 128    # 8 c-chunks
    TJ = T // 512    # 4 t-chunks of 512
    scale = 1.0 / np.sqrt(DH)

    with tile.TileContext(nc) as tc:
        with tc.tile_pool(name="persist", bufs=1) as pp:
            # ---- persistent sbuf tensors -------------------------------
            qT = pp.tile([128, 2, T], BF16, tag="qT")   # [p, pair, t]
            kT = pp.tile([128, 2, T], BF16, tag="kT")
            vA = pp.tile([128, TC, HPC * (DH + 1)], BF16, tag="vA")
            yT = pp.tile([128, 2, T], BF16, tag="yT")
            wo_sb = pp.tile([128, 2, C], BF16, tag="wo")
            identb = pp.tile([128, 128], BF16, tag="identb")
            atrif = pp.tile([128, 128], BF16, tag="atrif")
            bqs = pp.tile([128, 2], F32, tag="bqs")
            bks = pp.tile([128, 2], F32, tag="bks")
            bvs = pp.tile([128, DQC], F32, tag="bvs")

            # constants
            make_identity(nc, identb[:])
            # atrif[p, f] = NEG where f < p (mask s>t inside a diagonal
            # block); accumulated into PSUM via identb-stationary matmul
            make_lower_triangular(nc, atrif[:], val=NEG, diag=False)
            # ones column of v_aug
            vA4 = vA[:].rearrange("p s (h d) -> p s h d", d=DH + 1)
            onesf = pp.tile([128, TC * HPC], F32, tag="onesf")
            nc.gpsimd.memset(onesf[:], 1.0)
            nc.vector.tensor_copy(
                vA4[:, :, :, DH : DH + 1],
                onesf[:].rearrange("p (s h o) -> p s h o", h=HPC, o=1),
            )

            # biases
            nc.sync.dma_start(bqs[:], bq.ap().rearrange("(k p) -> p k", p=128))
            nc.vector.tensor_scalar_mul(bqs[:], bqs[:], scale)
            nc.sync.dma_start(bks[:], bk.ap().rearrange("(k p) -> p k", p=128))
            nc.sync.dma_start(
                bvs[0:1, :], bv.ap().rearrange("(o n) -> o n", o=1)
            )
            nc.gpsimd.partition_broadcast(bvs[:], bvs[0:1, :])

            # ---- phase 0/P: projections (scoped pools) -----------------
            with (
                tc.tile_pool(name="xw", bufs=1) as pw,
                tc.tile_pool(name="ps_misc", bufs=2, space="PSUM") as ps_misc,
            ):
                xT = pw.tile([128, CC, T], BF16, tag="xT")
                wq_sb = pw.tile([128, CC, DQC], BF16, tag="wq")
                wk_sb = pw.tile([128, CC, DQC], BF16, tag="wk")
                wv_sb = pw.tile([128, CC, DQC], BF16, tag="wv")

                # x^T streamed straight from HBM (host pre-transposed)
                # in tj-sized chunks; weights for the first projections go
                # first so the PE can start at ~chunk-0 landing
                xTr = xbT.ap().rearrange("(c p) t -> p c t", p=128)
                for tj in range(TJ):
                    nc.sync.dma_start(
                        xT[:, :, ts(tj, 512)], xTr[:, :, ts(tj, 512)]
                    )
                nc.scalar.dma_start(
                    wq_sb[:], wq.ap().rearrange("(c p) d -> p c d", p=128)
                )
                nc.scalar.dma_start(
                    wk_sb[:], wk.ap().rearrange("(c p) d -> p c d", p=128)
                )
                nc.scalar.dma_start(
                    wv_sb[:], wv.ap().rearrange("(c p) d -> p c d", p=128)
                )
                nc.scalar.dma_start(
                    wo_sb[:], wo.ap().rearrange("(k p) n -> p k n", p=128)
                )

                # Q^T / K^T / V projections, tj-outer so each x^T
                # chunk is consumed as soon as its DMA lands
                for tj in range(TJ):
                    for hp in range(2):
                        pq = ps_misc.tile([128, 512], F32, tag="misc")
                        for cc in range(CC):
                            nc.tensor.matmul(
                                pq[:],
                                wq_sb[:, cc, ts(hp, 128)],
                                xT[:, cc, ts(tj, 512)],
                                start=(cc == 0),
                                stop=(cc == CC - 1),
                            )
                        nc.vector.tensor_scalar(
                            qT[:, hp, ts(tj, 512)],
                            pq[:],
                            scale,
                            bqs[:, hp : hp + 1],
                            ALU.mult,
                            ALU.add,
                        )
                        pk = ps_misc.tile([128, 512], F32, tag="misc")
                        for cc in range(CC):
                            nc.tensor.matmul(
                                pk[:],
                                wk_sb[:, cc, ts(hp, 128)],
                                xT[:, cc, ts(tj, 512)],
                                start=(cc == 0),
                                stop=(cc == CC - 1),
                            )
                        nc.vector.tensor_scalar(
                            kT[:, hp, ts(tj, 512)],
                            pk[:],
                            bks[:, hp : hp + 1],
                            None,
                            ALU.add,
                        )
                    for sc in range(4 * tj, 4 * tj + 4):
                        pv = ps_misc.tile([128, 512], F32, tag="misc")
                        for cc in range(CC):
                            nc.tensor.matmul(
                                pv[:, :DQC],
                                xT[:, cc, ts(sc, 128)],
                                wv_sb[:, cc, :],
                                start=(cc == 0),
                                stop=(cc == CC - 1),
                            )
                        nc.vector.tensor_tensor(
                            vA4[:, sc, :, :DH],
                            pv[:, :DQC].rearrange("p (h d) -> p h d", d=DH),
                            bvs[:].rearrange("p (h d) -> p h d", d=DH),
                            ALU.add,
                        )

            # ---- phase A: attention + o-projection ---------------------
            with (
                tc.tile_pool(name="psb", bufs=4) as pexp,
                tc.tile_pool(name="small", bufs=6) as psm,
                tc.tile_pool(name="outp", bufs=3) as pout,
                tc.tile_pool(name="ps_s", bufs=2, space="PSUM") as ps_s,
                tc.tile_pool(name="ps_pv", bufs=3, space="PSUM") as ps_pv,
                tc.tile_pool(name="ps_tr", bufs=1, space="PSUM") as ps_tr,
            ):
                def oproj(tj):
                    # o-projection for t-chunk tj (consumes yT)
                    for tt in range(4):
                        t0 = 512 * tj + 128 * tt
                        ot = pout.tile([128, C], BF16, tag="o")
                        for nb in range(2):
                            po = ps_pv.tile([128, 4, 128], F32, tag="pv")
                            pof = po[:].rearrange("p a b -> p (a b)")
                            for kk in range(2):
                                nc.tensor.matmul(
                                    pof,
                                    yT[:, kk, ds(t0, 128)],
                                    wo_sb[:, kk, ts(nb, 512)],
                                    start=(kk == 0),
                                    stop=(kk == 1),
                                )
                            nc.any.tensor_copy(ot[:, ts(nb, 512)], pof)
                        nc.sync.dma_start(out.ap()[ds(t0, 128), :], ot[:])

                for tj in range(TJ):
                    n_sc = 4 * (tj + 1)
                    yNs = {}
                    for hp in range(2):
                        hA, hB = 2 * hp, 2 * hp + 1
                        # ppv[t-part, tb, 0:DH+1]: y_nat accumulators + den col
                        ppv_A = ps_pv.tile([128, 4, 128], F32, tag="pv")
                        ppv_B = ps_pv.tile([128, 4, 128], F32, tag="pv")
                        for sc in range(n_sc):
                            k = sc - 4 * tj  # >=0 on the causal diagonal
                            off = 128 * k if k > 0 else 0
                            pss = ps_s.tile([128, 1024], F32, tag="s")
                            # QK^T for both heads of the pair, row-packed;
                            # only columns [off, 512) are live
                            for hi, (half, ppos) in enumerate(
                                [(0, (0, 0)), (512, (64, 0))]
                            ):
                                prow = slice(64 * hi, 64 * hi + 64)
                                nc.tensor.matmul(
                                    pss[:, ds(half + off, 512 - off)],
                                    kT[prow, hp, ts(sc, 128)],
                                    qT[prow, hp, ds(512 * tj + off, 512 - off)],
                                    start=True,
                                    stop=True,
                                    tile_position=ppos,
                                    skip_group_check=(k >= 0),
                                )
                                if k >= 0:
                                    # causal mask on the diagonal 128-block
                                    # accumulated by the PE
                                    nc.tensor.matmul(
                                        pss[:, ds(half + off, 128)],
                                        identb[:],
                                        atrif[:],
                                        start=False,
                                        stop=True,
                                        skip_group_check=True,
                                    )
                            psb = pexp.tile([128, 1024], BF16, tag="p")
                            if off:
                                nc.scalar.activation(
                                    psb[:]
                                    .rearrange("p (h t) -> p h t", h=2)[
                                        :, :, off:
                                    ],
                                    pss[:]
                                    .rearrange("p (h t) -> p h t", h=2)[
                                        :, :, off:
                                    ],
                                    AF.Exp,
                                )
                            else:
                                nc.scalar.activation(psb[:], pss[:], AF.Exp)
                            # flipped PV: psb block is the stationary
                            # ([s,t]-block = lhsT), vA the 65-row moving ->
                            # y accumulates in natural [t, d] layout with the
                            # denominator in column DH
                            for hi, h in ((0, hA), (1, hB)):
                                ppv = ppv_A if hi == 0 else ppv_B
                                for tb in range(max(k, 0), 4):
                                    nc.tensor.matmul(
                                        ppv[:, tb, : DH + 1],
                                        psb[:, ds(512 * hi + 128 * tb, 128)],
                                        vA[:, sc, ds(h * (DH + 1), DH + 1)],
                                        start=(sc == 0 and tb == 0),
                                        stop=(sc == 4 * tj + tb),
                                        skip_group_check=True,
                                    )
                        # normalize: y = y_unnorm * (1/den) with per-partition
                        # (t) denominators - no cross-partition broadcast
                        for hi, h in ((0, hA), (1, hB)):
                            ppv = ppv_A if hi == 0 else ppv_B
                            dc = psm.tile([128, 4], F32, tag="dc")
                            nc.vector.tensor_copy(
                                dc[:],
                                ppv[:, :, DH : DH + 1].rearrange(
                                    "p a o -> p (a o)"
                                ),
                            )
                            rc = psm.tile([128, 4], F32, tag="rc")
                            sc2 = psm.tile([128, 4], F32, tag="sc2")
                            nc.vector.reciprocal_approx_accurate(
                                rc[:], dc[:], sc2[:]
                            )
                            yN = psm.tile([128, 4, DH], BF16, tag="yn")
                            for tb in range(4):
                                nc.vector.tensor_scalar(
                                    yN[:, tb, :],
                                    ppv[:, tb, :DH],
                                    rc[:, tb : tb + 1],
                                    None,
                                    ALU.mult,
                                )
                            yNs[hp, hi] = yN
                    # transpose y back to [dq, t] for the o-projection;
                    # hp0's transposes are ready first, then the deferred
                    # o-projection fills PE time while hp1's normalize runs
                    for hp in range(2):
                        if hp == 1 and tj > 0:
                            oproj(tj - 1)
                        ytr = ps_tr.tile([128, 4, 128], BF16, tag="tr")
                        for hi in range(2):
                            yN = yNs[hp, hi]
                            for tb in range(4):
                                nc.tensor.transpose(
                                    ytr[ds(64 * hi, 64), tb, :],
                                    yN[:, tb, :],
                                    identb[:],
                                    tile_position=(0, 64 * hi),
                                )
                        nc.any.tensor_copy(
                            yT[:, hp, ts(tj, 512)],
                            ytr[:].rearrange("p a b -> p (a b)"),
                        )
                oproj(TJ - 1)

    nc.compile()
    return nc


_CACHE = {}


def _get_program():
    if "nc" not in _CACHE:
        _CACHE["nc"] = build_program()
    return _CACHE["nc"]


def make_in_maps(x, wq, bq, wk, bk, wv, bv, wo):
    bf = mybir.dt.np(BF16)
    xb_ = np.asarray(x, np.float32).astype(bf)
    wqb = np.asarray(wq, np.float32).astype(bf)
    wkb = np.asarray(wk, np.float32).astype(bf)
    wvb = np.asarray(wv, np.float32).astype(bf)
    wob = np.asarray(wo, np.float32).astype(bf)
    in_maps = []
    for core in range(N_CORES):
        b, g = core // 4, core % 4
        sl = slice(g * DQC, (g + 1) * DQC)
        in_maps.append(
            {
                "xbT": np.ascontiguousarray(xb_[b].T),
                "wq": np.ascontiguousarray(wqb[:, sl]),
                "wk": np.ascontiguousarray(wkb[:, sl]),
                "wv": np.ascontiguousarray(wvb[:, sl]),
                "wo": np.ascontiguousarray(wob[sl, :]),
                "bq": np.ascontiguousarray(np.asarray(bq, np.float32)[sl]),
                "bk": np.ascontiguousarray(np.asarray(bk, np.float32)[sl]),
                "bv": np.ascontiguousarray(np.asarray(bv, np.float32)[sl]),
            }
        )
    return in_maps


def kernel(x, wq, bq, wk, bk, wv, bv, wo, bo):
    from concourse import bass_utils

    bo = np.asarray(bo, dtype=np.float32)

    nc = _get_program()
    in_maps = make_in_maps(x, wq, bq, wk, bk, wv, bv, wo)
    res = bass_utils.run_bass_kernel_spmd(
        nc, in_maps, core_ids=list(range(N_CORES))
    )
    y = np.zeros((B, T, C), dtype=np.float32)
    for core in range(N_CORES):
        y[core // 4] += res.results[core]["out"]
    y += bo
    return y


# revision 23
# speedup vs baseline: 1.2118x; 1.2118x over previous
"""Multi-head masked attention on 8 Trainium2 NeuronCores.

Sharding: data-parallel over batch (B=2 -> 2 groups of 4 cores),
tensor-parallel over heads within a group (16 heads -> 4 heads/core).
Each core computes q/k/v projections for its 4 heads (column-sharded),
causal attention in the transposed (S^T) domain, and a row-sharded
partial o-projection. The host sums the 4 partials per batch element
and adds the output bias.

All matmul operands are bf16 (PSUM accumulation stays f32): bf16
draws less power than f32r under the HAM duty-cycle throttle. x is
pre-transposed on the host so x^T streams straight from HBM with no
PE transpose pass. The causal mask is accumulated into PSUM by the PE
(identity-stationary matmul streaming a triangular constant), and the
diagonal QK/exp work is restricted to the unmasked columns.

Self-contained: hardcodes shapes B=2, T=2048, C=1024, H=16, Dh=64.
"""

import sys

sys.path.insert(0, "/opt/trn_rl_repo")

import numpy as np

import concourse.bass as bass
import concourse.tile as tile
import concourse.mybir as mybir
from concourse import bacc
from concourse.bass import ts, ds
from concourse.masks import make_identity, make_lower_triangular

F32 = mybir.dt.float32
BF16 = mybir.dt.bfloat16
AF = mybir.ActivationFunctionType
ALU = mybir.AluOpType

B, T, C = 2, 2048, 1024
H, DH = 16, 64
HPC = 4            # heads per core
DQC = HPC * DH     # 256 projected dims per core
N_CORES = 8
NEG = -1.0e30


def build_program():
    nc = bacc.Bacc("TRN2", target_bir_lowering=False, debug=False)

    xbT = nc.dram_tensor("xbT", [C, T], BF16, kind="ExternalInput")
    wq = nc.dram_tensor("wq", [C, DQC], BF16, kind="ExternalInput")
    wk = nc.dram_tensor("wk", [C, DQC], BF16, kind="ExternalInput")
    wv = nc.dram_tensor("wv", [C, DQC], BF16, kind="ExternalInput")
    wo = nc.dram_tensor("wo", [DQC, C], BF16, kind="ExternalInput")
    bq = nc.dram_tensor("bq", [DQC], F32, kind="ExternalInput")
    bk = nc.dram_tensor("bk", [DQC], F32, kind="ExternalInput")
    bv = nc.dram_tensor("bv", [DQC], F32, kind="ExternalInput")
    out = nc.dram_tensor("out", [T, C], BF16, kind="ExternalOutput")

    TC = T // 128    # 16 t-chunks of 128
    CC = C // 128    # 8 c-chunks
    TJ = T // 512    # 4 t-chunks of 512
    scale = 1.0 / np.sqrt(DH)

    with tile.TileContext(nc) as tc:
        with tc.tile_pool(name="persist", bufs=1) as pp:
            # ---- persistent sbuf tensors -------------------------------
            qT = pp.tile([128, 2, T], BF16, tag="qT")   # [p, pair, t]
            kT = pp.tile([128, 2, T], BF16, tag="kT")
            vA = pp.tile([128, TC, HPC * (DH + 1)], BF16, tag="vA")
            yT = pp.tile([128, 2, T], BF16, tag="yT")
            wo_sb = pp.tile([128, 2, C], BF16, tag="wo")
            identb = pp.tile([128, 128], BF16, tag="identb")
            atrif = pp.tile([128, 128], BF16, tag="atrif")
            bqs = pp.tile([128, 2], F32, tag="bqs")
            bks = pp.tile([128, 2], F32, tag="bks")
            bvs = pp.tile([128, DQC], F32, tag="bvs")

            # constants
            make_identity(nc, identb[:])
            # atrif[p, f] = NEG where f < p (mask s>t inside a diagonal
            # block); accumulated into PSUM via identb-stationary matmul
            make_lower_triangular(nc, atrif[:], val=NEG, diag=False)
            # ones column of v_aug
            vA4 = vA[:].rearrange("p s (h d) -> p s h d", d=DH + 1)
            onesf = pp.tile([128, TC * HPC], F32, tag="onesf")
            nc.gpsimd.memset(onesf[:], 1.0)
            nc.vector.tensor_copy(
                vA4[:, :, :, DH : DH + 1],
                onesf[:].rearrange("p (s h o) -> p s h o", h=HPC, o=1),
            )

            # biases
            nc.sync.dma_start(bqs[:], bq.ap().rearrange("(k p) -> p k", p=128))
            nc.vector.tensor_scalar_mul(bqs[:], bqs[:], scale)
            nc.sync.dma_start(bks[:], bk.ap().rearrange("(k p) -> p k", p=128))
            nc.sync.dma_start(
                bvs[0:1, :], bv.ap().rearrange("(o n) -> o n", o=1)
            )
            nc.gpsimd.partition_broadcast(bvs[:], bvs[0:1, :])

            # ---- phase 0/P: projections (scoped pools) -----------------
            with (
                tc.tile_pool(name="xw", bufs=1) as pw,
                tc.tile_pool(name="ps_misc", bufs=2, space="PSUM") as ps_misc,
            ):
                xT = pw.tile([128, CC, T], BF16, tag="xT")
                wq_sb = pw.tile([128, CC, DQC], BF16, tag="wq")
                wk_sb = pw.tile([128, CC, DQC], BF16, tag="wk")
                wv_sb = pw.tile([128, CC, DQC], BF16, tag="wv")

                # x^T streamed straight from HBM (host pre-transposed)
                # in tj-sized chunks; weights for the first projections go
                # first so the PE can start at ~chunk-0 landing
                xTr = xbT.ap().rearrange("(c p) t -> p c t", p=128)
                nc.sync.dma_start(
                    wq_sb[:], wq.ap().rearrange("(c p) d -> p c d", p=128)
                )
                nc.sync.dma_start(
                    wk_sb[:], wk.ap().rearrange("(c p) d -> p c d", p=128)
                )
                nc.sync.dma_start(xT[:, :, ts(0, 512)], xTr[:, :, ts(0, 512)])
                nc.sync.dma_start(
                    wv_sb[:], wv.ap().rearrange("(c p) d -> p c d", p=128)
                )
                for tj in range(1, TJ):
                    nc.sync.dma_start(
                        xT[:, :, ts(tj, 512)], xTr[:, :, ts(tj, 512)]
                    )
                nc.sync.dma_start(
                    wo_sb[:], wo.ap().rearrange("(k p) n -> p k n", p=128)
                )

                # Q^T / K^T / V projections, tj-outer so each x^T
                # chunk is consumed as soon as its DMA lands
                for tj in range(TJ):
                    for hp in range(2):
                        pq = ps_misc.tile([128, 512], F32, tag="misc")
                        for cc in range(CC):
                            nc.tensor.matmul(
                                pq[:],
                                wq_sb[:, cc, ts(hp, 128)],
                                xT[:, cc, ts(tj, 512)],
                                start=(cc == 0),
                                stop=(cc == CC - 1),
                            )
                        nc.vector.tensor_scalar(
                            qT[:, hp, ts(tj, 512)],
                            pq[:],
                            scale,
                            bqs[:, hp : hp + 1],
                            ALU.mult,
                            ALU.add,
                        )
                        pk = ps_misc.tile([128, 512], F32, tag="misc")
                        for cc in range(CC):
                            nc.tensor.matmul(
                                pk[:],
                                wk_sb[:, cc, ts(hp, 128)],
                                xT[:, cc, ts(tj, 512)],
                                start=(cc == 0),
                                stop=(cc == CC - 1),
                            )
                        nc.vector.tensor_scalar(
                            kT[:, hp, ts(tj, 512)],
                            pk[:],
                            bks[:, hp : hp + 1],
                            None,
                            ALU.add,
                        )
                    for sc in range(4 * tj, 4 * tj + 4):
                        pv = ps_misc.tile([128, 512], F32, tag="misc")
                        for cc in range(CC):
                            nc.tensor.matmul(
                                pv[:, :DQC],
                                xT[:, cc, ts(sc, 128)],
                                wv_sb[:, cc, :],
                                start=(cc == 0),
                                stop=(cc == CC - 1),
                            )
                        nc.vector.tensor_tensor(
                            vA4[:, sc, :, :DH],
                            pv[:, :DQC].rearrange("p (h d) -> p h d", d=DH),
                            bvs[:].rearrange("p (h d) -> p h d", d=DH),
                            ALU.add,
                        )

            # ---- phase A: attention + o-projection ---------------------
            with (
                tc.tile_pool(name="psb", bufs=3) as pexp,
                tc.tile_pool(name="small", bufs=4) as psm,
                tc.tile_pool(name="outp", bufs=3) as pout,
                tc.tile_pool(name="ps_s", bufs=2, space="PSUM") as ps_s,
                tc.tile_pool(name="ps_pv", bufs=3, space="PSUM") as ps_pv,
                tc.tile_pool(name="ps_tr", bufs=1, space="PSUM") as ps_tr,
            ):
                def oproj(tj):
                    # o-projection for t-chunk tj (consumes yT)
                    for tt in range(4):
                        t0 = 512 * tj + 128 * tt
                        ot = pout.tile([128, C], BF16, tag="o")
                        for nb in range(2):
                            po = ps_pv.tile([128, 4, 128], F32, tag="pv")
                            pof = po[:].rearrange("p a b -> p (a b)")
                            for kk in range(2):
                                nc.tensor.matmul(
                                    pof,
                                    yT[:, kk, ds(t0, 128)],
                                    wo_sb[:, kk, ts(nb, 512)],
                                    start=(kk == 0),
                                    stop=(kk == 1),
                                )
                            nc.any.tensor_copy(ot[:, ts(nb, 512)], pof)
                        nc.sync.dma_start(out.ap()[ds(t0, 128), :], ot[:])

                for tj in range(TJ):
                    n_sc = 4 * (tj + 1)
                    yNs = {}
                    for hp in range(2):
                        hA, hB = 2 * hp, 2 * hp + 1
                        # ppv[t-part, tb, 0:DH+1]: y_nat accumulators + den col
                        ppv_A = ps_pv.tile([128, 4, 128], F32, tag="pv")
                        ppv_B = ps_pv.tile([128, 4, 128], F32, tag="pv")
                        for sc in range(n_sc):
                            k = sc - 4 * tj  # >=0 on the causal diagonal
                            off = 128 * k if k > 0 else 0
                            pss = ps_s.tile([128, 1024], F32, tag="s")
                            # QK^T for both heads of the pair, row-packed;
                            # only columns [off, 512) are live
                            for hi, (half, ppos) in enumerate(
                                [(0, (0, 0)), (512, (64, 0))]
                            ):
                                prow = slice(64 * hi, 64 * hi + 64)
                                nc.tensor.matmul(
                                    pss[:, ds(half + off, 512 - off)],
                                    kT[prow, hp, ts(sc, 128)],
                                    qT[prow, hp, ds(512 * tj + off, 512 - off)],
                                    start=True,
                                    stop=True,
                                    tile_position=ppos,
                                    skip_group_check=(k >= 0),
                                )
                                if k >= 0:
                                    # causal mask on the diagonal 128-block
                                    # accumulated by the PE
                                    nc.tensor.matmul(
                                        pss[:, ds(half + off, 128)],
                                        identb[:],
                                        atrif[:],
                                        start=False,
                                        stop=True,
                                        skip_group_check=True,
                                    )
                            psb = pexp.tile([128, 1024], BF16, tag="p")
                            if off:
                                nc.scalar.activation(
                                    psb[:]
                                    .rearrange("p (h t) -> p h t", h=2)[
                                        :, :, off:
                                    ],
                                    pss[:]
                                    .rearrange("p (h t) -> p h t", h=2)[
                                        :, :, off:
                                    ],
                                    AF.Exp,
                                )
                            else:
                                nc.scalar.activation(psb[:], pss[:], AF.Exp)
                            # flipped PV: psb block is the stationary
                            # ([s,t]-block = lhsT), vA the 65-row moving ->
                            # y accumulates in natural [t, d] layout with the
                            # denominator in column DH
                            for hi, h in ((0, hA), (1, hB)):
                                ppv = ppv_A if hi == 0 else ppv_B
                                for tb in range(max(k, 0), 4):
                                    nc.tensor.matmul(
                                        ppv[:, tb, : DH + 1],
                                        psb[:, ds(512 * hi + 128 * tb, 128)],
                                        vA[:, sc, ds(h * (DH + 1), DH + 1)],
                                        start=(sc == 0 and tb == 0),
                                        stop=(sc == 4 * tj + tb),
                                        skip_group_check=True,
                                    )
                        # normalize: y = y_unnorm * (1/den) with per-partition
                        # (t) denominators - no cross-partition broadcast
                        for hi, h in ((0, hA), (1, hB)):
                            ppv = ppv_A if hi == 0 else ppv_B
                            dc = psm.tile([128, 4], F32, tag="dc")
                            nc.vector.tensor_copy(
                                dc[:],
                                ppv[:, :, DH : DH + 1].rearrange(
                                    "p a o -> p (a o)"
                                ),
                            )
                            rc = psm.tile([128, 4], F32, tag="rc")
                            sc2 = psm.tile([128, 4], F32, tag="sc2")
                            nc.vector.reciprocal_approx_accurate(
                                rc[:], dc[:], sc2[:]
                            )
                            yN = psm.tile([128, 4, DH], BF16, tag="yn")
                            for tb in range(4):
                                nc.vector.tensor_scalar(
                                    yN[:, tb, :],
                                    ppv[:, tb, :DH],
                                    rc[:, tb : tb + 1],
                                    None,
                                    ALU.mult,
                                )
                            yNs[hp, hi] = yN
                    if tj > 0:
                        oproj(tj - 1)
                    # transpose y back to [dq, t] for the o-projection
                    for hp in range(2):
                        ytr = ps_tr.tile([128, 4, 128], BF16, tag="tr")
                        for hi in range(2):
                            yN = yNs[hp, hi]
                            for tb in range(4):
                                nc.tensor.transpose(
                                    ytr[ds(64 * hi, 64), tb, :],
                                    yN[:, tb, :],
                                    identb[:],
                                    tile_position=(0, 64 * hi),
                                )
                        nc.any.tensor_copy(
                            yT[:, hp, ts(tj, 512)],
                            ytr[:].rearrange("p a b -> p (a b)"),
                        )
                oproj(TJ - 1)

    nc.compile()
    return nc


_CACHE = {}


def _get_program():
    if "nc" not in _CACHE:
        _CACHE["nc"] = build_program()
    return _CACHE["nc"]


def make_in_maps(x, wq, bq, wk, bk, wv, bv, wo):
    bf = mybir.dt.np(BF16)
    xb_ = np.asarray(x, np.float32).astype(bf)
    wqb = np.asarray(wq, np.float32).astype(bf)
    wkb = np.asarray(wk, np.float32).astype(bf)
    wvb = np.asarray(wv, np.float32).astype(bf)
    wob = np.asarray(wo, np.float32).astype(bf)
    in_maps = []
    for core in range(N_CORES):
        b, g = core // 4, core % 4
        sl = slice(g * DQC, (g + 1) * DQC)
        in_maps.append(
            {
                "xbT": np.ascontiguousarray(xb_[b].T),
                "wq": np.ascontiguousarray(wqb[:, sl]),
                "wk": np.ascontiguousarray(wkb[:, sl]),
                "wv": np.ascontiguousarray(wvb[:, sl]),
                "wo": np.ascontiguousarray(wob[sl, :]),
                "bq": np.ascontiguousarray(np.asarray(bq, np.float32)[sl]),
                "bk": np.ascontiguousarray(np.asarray(bk, np.float32)[sl]),
                "bv": np.ascontiguousarray(np.asarray(bv, np.float32)[sl]),
            }
        )
    return in_maps


def kernel(x, wq, bq, wk, bk, wv, bv, wo, bo):
    from concourse import bass_utils

    bo = np.asarray(bo, dtype=np.float32)

    nc = _get_program()
    in_maps = make_in_maps(x, wq, bq, wk, bk, wv, bv, wo)
    res = bass_utils.run_bass_kernel_spmd(
        nc, in_maps, core_ids=list(range(N_CORES))
    )
    y = np.zeros((B, T, C), dtype=np.float32)
    for core in range(N_CORES):
        y[core // 4] += res.results[core]["out"]
    y += bo
    return y
